# revision 32
# baseline (speedup 1.0000x reference)
"""AdaptiveTokenSampling Trainium2 kernel (8 NeuronCores, SPMD, no collectives).

Sharding: core c handles batch b = c//2 and heads h in [half*8, half*8+8),
half = c%2. The sampling prefix (entropy-weighted cls scores -> gumbel argmax
-> sorted-unique token ids) is replicated on both cores of a b-pair: it is
cheap (~5MB of reads) and replicating it removes every cross-core collective.
The heavy part - gathering 257 rows of attn per (b,h) slice - is done with
indirect DMA using gather indices constructed fully on-chip.

All integer-valued intermediates (onehot indicators, slot ids, token ids
<= 2048) ride fp16 tensors: exact, and fp16 matmuls are single-pass on PE
where f32 needs LOW/HIGH double passes.

Per-core inputs (host shards/replicates):
  attn_s  (8, 1024, 1024) f32  attn[b, half*8:(half+1)*8]
  value_s (16, 1024, 64)  f32  value[b]
  cls_s   (16, 1023)      f32  attn[b, :, 0, 1:]
  u_s     (256, 1023)     f32  u[b]
Per-core outputs:
  out  (2056, 1024) f32  gathered rows, slice-major (8 slices x 257 rows)
  uniq (1, 257)     i32  unique sampled token ids (zero-padded, cls first)
"""

import os
import numpy as np

N_CORES = 8
N = 1024
W = 1023          # n - 1
K = 256           # OUTPUT_NUM_TOKENS
K1 = 257          # K + 1 (cls prepended)
NSL = 16          # slices (heads) per b for the sampling prefix
NGS = 8           # gather slices per core
EPS = 1e-6

_CACHE = {}


def _build(stage=4, debug=False):
    import concourse.bacc as bacc
    import concourse.bass as bass
    import concourse.mybir as mybir
    import concourse.tile as tile

    f32 = mybir.dt.float32
    bf16 = mybir.dt.bfloat16
    f16 = mybir.dt.float16
    i32 = mybir.dt.int32
    u32 = mybir.dt.uint32
    Alu = mybir.AluOpType
    Act = mybir.ActivationFunctionType

    nc = bacc.Bacc("TRN2", target_bir_lowering=False, debug=False,
                   num_devices=N_CORES)

    attn_s = nc.dram_tensor("attn_s", [NGS, N, N], f32, kind="ExternalInput")
    value_s = nc.dram_tensor("value_s", [NSL, N, 64], f32, kind="ExternalInput")
    cls_s = nc.dram_tensor("cls_s", [NSL, W], f32, kind="ExternalInput")
    u_s = nc.dram_tensor("u_s", [K, W], f32, kind="ExternalInput")
    out_g = nc.dram_tensor("out", [NGS * K1, N], f32, kind="ExternalOutput")
    uniq_o = nc.dram_tensor("uniq", [1, K1], i32, kind="ExternalOutput")
    if debug:
        dbg_o = nc.dram_tensor("dbg", [128, W], f32, kind="ExternalOutput")

    attn_flat = attn_s.ap().rearrange("s t w -> (s t) w")

    with tile.TileContext(nc) as tc:
        with (
            tc.tile_pool(name="sb", bufs=1) as sb,
            tc.tile_pool(name="sbv", bufs=2) as sbv,
            tc.tile_pool(name="sbu", bufs=2) as sbu,
            tc.tile_pool(name="sbg", bufs=8) as sbg,
            tc.tile_pool(name="ps", bufs=1, space="PSUM") as ps,
        ):
            # ---- input loads first: keep the DMA rings busy from t=0 ----
            # value in 4 flat 1MB chunks (8KB/partition descriptors, full BW).
            # chunk c, partition p, col x: slice = 4c + p//32,
            # token = (p%32)*32 + x//64, d = x%64.
            v_flat = value_s.ap().rearrange("s t d -> (s t d)")
            vchs = []
            for c in range(4):
                vch = sbv.tile([128, 2048], f32, tag="vch", bufs=4)
                eng = nc.sync if c % 2 == 0 else nc.scalar
                eng.dma_start(vch[:], v_flat[c * 262144:(c + 1) * 262144]
                              .rearrange("(p x) -> p x", p=128))
                vchs.append(vch)
            cls_rows = sb.tile([NSL, W], f32)
            nc.scalar.dma_start(cls_rows[:], cls_s.ap())
            u_sbs = []
            for kc in range(2):
                u_sb = sbu.tile([128, W], f32, tag="u")
                eng = nc.sync if kc == 0 else nc.scalar
                eng.dma_start(u_sb[:], u_s.ap()[kc * 128:(kc + 1) * 128])
                u_sbs.append(u_sb)

            # ---- constants ----
            bias9 = sb.tile([128, 1], f32)
            nc.vector.memset(bias9[:], 1e-9)
            bias6 = sb.tile([128, 1], f32)
            nc.vector.memset(bias6[:], EPS)
            ones11_16 = sb.tile([1, 1], f16)
            nc.vector.memset(ones11_16[:], 1.0)
            ones_128_1_16 = sb.tile([128, 1], f16)
            nc.vector.memset(ones_128_1_16[:], 1.0)
            ones_16_128 = sb.tile([16, 128], f16)
            nc.vector.memset(ones_16_128[:], 1.0)
            ones_8_16 = sb.tile([1, 8], f16)
            nc.vector.memset(ones_8_16[:], 1.0)
            iota1024 = sb.tile([128, N], f16)
            nc.gpsimd.iota(iota1024[:], pattern=[[1, N]], base=-1,
                           channel_multiplier=0,
                           allow_small_or_imprecise_dtypes=True)
            iota257 = sb.tile([128, K1], f16)
            nc.gpsimd.iota(iota257[:], pattern=[[1, K1]], base=1,
                           channel_multiplier=0,
                           allow_small_or_imprecise_dtypes=True)
            tvals16 = sb.tile([128, 8], f16)
            nc.gpsimd.iota(tvals16[:], pattern=[[128, 8]], base=0,
                           channel_multiplier=1,
                           allow_small_or_imprecise_dtypes=True)
            rowbase8 = sb.tile([8, 1], f32)
            nc.gpsimd.iota(rowbase8[:], pattern=[[1, 1]], base=0,
                           channel_multiplier=1024,
                           allow_small_or_imprecise_dtypes=True)
            # tok0_mask zeroes each slice's token-0 entropy term: those sit at
            # (p in {0,32,64,96}, col in {0,32,64,96}) of the (128,128) w grid
            tok0_mask = sb.tile([128, NSL * 8], f32)
            nc.vector.memset(tok0_mask[:], 1.0)
            for pp in range(0, 128, 32):
                for cc in range(0, 128, 32):
                    nc.vector.memset(tok0_mask[pp:pp + 1, cc:cc + 1], 0.0)
            # ssel_c[p, s] = 1 iff slice(chunk c, p) == s
            ssels = []
            for c in range(4):
                ssel = sb.tile([128, NSL], f32, name=f"ssel{c}", tag=f"ssel{c}")
                nc.vector.memset(ssel[:], 0.0)
                for h in range(4):
                    nc.vector.memset(
                        ssel[32 * h:32 * h + 32, 4 * c + h:4 * c + h + 1], 1.0)
                ssels.append(ssel)

            # ---- stage 1: value norms -> entropy -> pseudo logits ----
            vn2 = sb.tile([128, NSL * 8], f32)
            for c in range(4):
                vsq = sbv.tile([128, 2048], bf16, tag="vsq")
                if c % 2 == 0:
                    nc.scalar.square(vsq[:], vchs[c][:])
                else:
                    nc.vector.tensor_mul(vsq[:], vchs[c][:], vchs[c][:])
                nc.vector.tensor_reduce(
                    out=vn2[:, c * 32:(c + 1) * 32],
                    in_=vsq[:].rearrange("p (tk d) -> p tk d", d=64),
                    axis=mybir.AxisListType.X, op=Alu.add)
            vn = sb.tile([128, NSL * 8], f32)
            nc.scalar.sqrt(vn[:], vn2[:])
            lnvn = sb.tile([128, NSL * 8], f32)
            nc.scalar.activation(lnvn[:], vn[:], Act.Ln, bias=bias9[:, 0:1])
            w_all = sb.tile([128, NSL * 8], f32)
            nc.vector.tensor_mul(w_all[:], vn[:], lnvn[:])
            wm = sb.tile([128, NSL * 8], f32)
            nc.vector.tensor_mul(wm[:], w_all[:], tok0_mask[:])
            w_red = sb.tile([128, 4], f32)
            nc.vector.tensor_reduce(
                out=w_red[:],
                in_=wm[:].rearrange("p (c j) -> p c j", j=32),
                axis=mybir.AxisListType.X, op=Alu.add)
            # ent_ps = -entropy; the sign cancels in normed = cls/(sum+eps)
            ent_ps = ps.tile([16, 1], f32, tag="A")
            for c in range(4):
                nc.tensor.matmul(out=ent_ps[:], lhsT=ssels[c][:],
                                 rhs=w_red[:, c:c + 1],
                                 start=(c == 0), stop=(c == 3))

            # fp16 weighted: |values| <= ~17k fits fp16 range; the resulting
            # ~2e-4 relative pl error is 12x below the minimum argmax margin,
            # and the fp16 cls matmul is single-pass on PE (f32 is double).
            weighted = sb.tile([NSL, W], f16)
            nc.vector.tensor_scalar(out=weighted[:], in0=cls_rows[:],
                                    scalar1=ent_ps[:], scalar2=None,
                                    op0=Alu.mult)
            # lhsT=ones(16,128) replicates the cls row into all 128 PSUM
            # partitions at no extra PE cost - the later gumbel subtract
            # needs it broadcast anyway.
            cls_ps = ps.tile([128, W], f32, tag="B")
            nc.tensor.matmul(out=cls_ps[:, 0:512], lhsT=ones_16_128[:],
                             rhs=weighted[:, 0:512], start=True, stop=True)
            nc.tensor.matmul(out=cls_ps[:, 512:W], lhsT=ones_16_128[:],
                             rhs=weighted[:, 512:W], start=True, stop=True)
            # S + 1e-6 == S in f32 (|S| ~ 1e8), so the eps add is dropped;
            # normed is fused into the Ln via the per-partition scale operand.
            s_sb = sb.tile([128, 1], f32)
            nc.vector.reduce_sum(out=s_sb[:], in_=cls_ps[:, :],
                                 axis=mybir.AxisListType.X)
            r_sb = sb.tile([128, 1], f32)
            nc.vector.reciprocal(r_sb[:], s_sb[:])
            pl = sb.tile([128, W], f32)
            nc.scalar.activation(pl[:], cls_ps[:, :], Act.Ln,
                                 bias=bias6[:, 0:1], scale=r_sb[:, 0:1])

            if debug and stage == 1:
                nc.sync.dma_start(dbg_o.ap()[0:1, :], pl[0:1, :])

            if stage >= 2:
                # ---- gumbel perturbation + argmax (2 chunks of 128) ----
                ids_f32 = sb.tile([128, 2], f32)
                for kc in range(2):
                    t1 = sbu.tile([128, W], f32, tag="t1")
                    nc.scalar.activation(t1[:], u_sbs[kc][:], Act.Ln,
                                         bias=bias6[:, 0:1])
                    t2 = sbu.tile([128, W], f32, tag="t2")
                    nc.scalar.activation(t2[:], t1[:], Act.Ln,
                                         bias=bias6[:, 0:1], scale=-1.0)
                    pert = sbu.tile([128, W], f32, tag="pert")
                    nc.vector.tensor_sub(pert[:], pl[:], t2[:])
                    mx8 = sbu.tile([128, 8], f32, tag="mx8")
                    nc.vector.max(mx8[:], pert[:])
                    ix8 = sbu.tile([128, 8], u32, tag="ix8")
                    nc.vector.max_index(ix8[:], mx8[:], pert[:])
                    nc.vector.tensor_copy(ids_f32[:, kc:kc + 1], ix8[:, 0:1])

                if debug and stage == 2:
                    nc.sync.dma_start(dbg_o.ap()[:, 0:2], ids_f32[:])

            if stage >= 3:
                # ---- presence counts (onehot matmul, fp16 single-pass) ----
                count_ps = ps.tile([1, N], f32, tag="B")
                for kc in range(2):
                    oh = sbu.tile([128, N], f16, tag="oh")
                    # gpsimd is idle here; taking the compares off the packed
                    # DVE chain lets them overlap the argmax max/find ops
                    nc.gpsimd.tensor_scalar(out=oh[:], in0=iota1024[:],
                                            scalar1=ids_f32[:, kc:kc + 1],
                                            scalar2=None, op0=Alu.is_equal)
                    for nh in range(2):
                        nc.tensor.matmul(
                            out=count_ps[0:1, nh * 512:(nh + 1) * 512],
                            lhsT=ones_128_1_16[:],
                            rhs=oh[:, nh * 512:(nh + 1) * 512],
                            start=(kc == 0), stop=(kc == 1))

                presence = sb.tile([1, N], f32)
                nc.vector.tensor_scalar(out=presence[:], in0=count_ps[0:1, :],
                                        scalar1=0.0, scalar2=None,
                                        op0=Alu.is_gt)
                pos = sb.tile([1, N], f32)
                nc.vector.tensor_tensor_scan(out=pos[:], data0=presence[:],
                                             data1=presence[:], initial=0.0,
                                             op0=Alu.add, op1=Alu.bypass)
                # q[t] = pos[t]+1 if present else 0; iota257 runs 1..257 so
                # slot j matches value j+1 and 0/absent never matches.
                q16 = sb.tile([1, N], f16)
                nc.vector.scalar_tensor_tensor(out=q16[:], in0=pos[:],
                                               scalar=1.0, in1=presence[:],
                                               op0=Alu.add, op1=Alu.mult)

                # ---- uniq via slot-match matmuls (all fp16, exact) ----
                qt_ps = ps.tile([128, 8], f32, tag="A")
                for tcch in range(8):
                    nc.tensor.matmul(
                        out=qt_ps[:, tcch:tcch + 1],
                        lhsT=q16[0:1, tcch * 128:(tcch + 1) * 128],
                        rhs=ones11_16[:], start=True, stop=True)
                uniq_ps = ps.tile([1, K1], f32, tag="D")
                for tcch in range(8):
                    m_sb = sbu.tile([128, K1], f16, tag="m")
                    nc.vector.tensor_scalar(out=m_sb[:], in0=iota257[:],
                                            scalar1=qt_ps[:, tcch:tcch + 1],
                                            scalar2=None, op0=Alu.is_equal)
                    nc.tensor.matmul(out=uniq_ps[0:1, :],
                                     lhsT=tvals16[:, tcch:tcch + 1],
                                     rhs=m_sb[:],
                                     start=(tcch == 0), stop=(tcch == 7))
                uniq16 = sb.tile([1, K1], f16)
                nc.vector.tensor_copy(uniq16[:], uniq_ps[0:1, :])
                uniq_i32 = sb.tile([1, K1], i32)
                nc.vector.tensor_copy(uniq_i32[:], uniq_ps[0:1, :])
                nc.sync.dma_start(uniq_o.ap(), uniq_i32[:])

            if stage >= 4:
                # ---- gather indices ----
                ujt_ps = ps.tile([128, 2], f32, tag="E")
                for jc in range(2):
                    nc.tensor.matmul(
                        out=ujt_ps[:, jc:jc + 1],
                        lhsT=uniq16[0:1, jc * 128:(jc + 1) * 128],
                        rhs=ones11_16[:], start=True, stop=True)
                idx_sb = sb.tile([128, 2 * NGS], i32)
                for sg in range(NGS):
                    nc.vector.tensor_scalar(out=idx_sb[:, sg * 2:sg * 2 + 2],
                                            in0=ujt_ps[:],
                                            scalar1=float(sg * N),
                                            scalar2=None, op0=Alu.add)
                # broadcast uniq[256] to 8 partitions via rank-1 matmul
                stb_ps = ps.tile([8, 1], f32, tag="A")
                nc.tensor.matmul(out=stb_ps[:], lhsT=ones_8_16[:],
                                 rhs=uniq16[0:1, 256:257],
                                 start=True, stop=True)
                st_idx = sb.tile([8, 1], i32)
                nc.vector.tensor_scalar(out=st_idx[:], in0=stb_ps[:],
                                        scalar1=rowbase8[:], scalar2=None,
                                        op0=Alu.add)

                # ---- the gathers (HW indirect DMA needs (P,1) offsets) ----
                st_tile = sbg.tile([8, N], f32)
                nc.gpsimd.indirect_dma_start(
                    out=st_tile[:],
                    out_offset=None,
                    in_=attn_flat,
                    in_offset=bass.IndirectOffsetOnAxis(ap=st_idx[:], axis=0),
                )
                st_dst = out_g.ap().rearrange("(s j) w -> s j w", j=K1)[:, 256]
                nc.sync.dma_start(st_dst, st_tile[:])
                for sg in range(NGS):
                    for jc in range(2):
                        g_tile = sbg.tile([128, N], f32, tag="g")
                        nc.gpsimd.indirect_dma_start(
                            out=g_tile[:],
                            out_offset=None,
                            in_=attn_flat,
                            in_offset=bass.IndirectOffsetOnAxis(
                                ap=idx_sb[:, sg * 2 + jc:sg * 2 + jc + 1],
                                axis=0),
                        )
                        r0 = sg * K1 + jc * 128
                        eng = nc.sync if (sg * 2 + jc) % 2 == 0 else nc.scalar
                        eng.dma_start(out_g.ap()[r0:r0 + 128], g_tile[:])

    nc.compile()
    return nc


def _get_nc():
    stage = int(os.environ.get("ATS_STAGE", "4"))
    debug = bool(int(os.environ.get("ATS_DEBUG", "0")))
    key = ("nc", stage, debug)
    if key not in _CACHE:
        _CACHE[key] = _build(stage, debug)
    return _CACHE[key]


def kernel(attn, value, u, mask=None):
    from concourse.bass_utils import run_bass_kernel_spmd

    attn = np.ascontiguousarray(attn, dtype=np.float32)
    value = np.ascontiguousarray(value, dtype=np.float32)
    u = np.ascontiguousarray(u, dtype=np.float32)

    nc = _get_nc()
    in_maps = []
    for c in range(N_CORES):
        b, half = c // 2, c % 2
        in_maps.append({
            "attn_s": np.ascontiguousarray(attn[b, half * 8:(half + 1) * 8]),
            "value_s": value[b],
            "cls_s": np.ascontiguousarray(attn[b, :, 0, 1:]),
            "u_s": u[b],
        })
    trace = bool(int(os.environ.get("ATS_TRACE", "0")))
    res = None
    for attempt in range(3):
        try:
            res = run_bass_kernel_spmd(nc, in_maps,
                                       core_ids=list(range(N_CORES)),
                                       trace=trace)
            break
        except Exception:
            # transient NRT/device hiccups (e.g. a prior crashed session
            # left the accelerator recovering) usually clear on retry
            if attempt == 2:
                raise
            import time
            time.sleep(15)
    kernel.last_exec_ns = res.exec_time_ns
    kernel.last_results = res.results

    new_attn = np.empty((4, 16, K1, N), np.float32)
    uniq = np.empty((4, K1), np.int32)
    for c in range(N_CORES):
        b, half = c // 2, c % 2
        new_attn[b, half * 8:(half + 1) * 8] = (
            res.results[c]["out"].reshape(NGS, K1, N))
        if half == 0:
            uniq[b] = res.results[c]["uniq"][0]
    new_mask = uniq != 0
    new_mask[:, 0] = True
    return new_attn, new_mask, uniq


kernel.last_exec_ns = None
kernel.last_results = None


# revision 33
# speedup vs baseline: 1.2612x; 1.2612x over previous
"""AdaptiveTokenSampling Trainium2 kernel (8 NeuronCores, SPMD, no collectives).

Sharding: core c handles batch b = c//2 and heads h in [half*8, half*8+8),
half = c%2. The sampling prefix (entropy-weighted cls scores -> gumbel argmax
-> sorted-unique token ids) is replicated on both cores of a b-pair: it is
cheap (~5MB of reads) and replicating it removes every cross-core collective.
The heavy part - gathering 257 rows of attn per (b,h) slice - is done with
indirect DMA using gather indices constructed fully on-chip.

All integer-valued intermediates (onehot indicators, slot ids, token ids
<= 2048) ride fp16 tensors: exact, and fp16 matmuls are single-pass on PE
where f32 needs LOW/HIGH double passes.

Per-core inputs (host shards/replicates):
  attn_s  (8, 1024, 1024) f32  attn[b, half*8:(half+1)*8]
  value_s (16, 1024, 64)  f32  value[b]
  cls_s   (16, 1023)      f32  attn[b, :, 0, 1:]
  u_s     (256, 1023)     f32  u[b]
Per-core outputs:
  out  (2056, 1024) f32  gathered rows, slice-major (8 slices x 257 rows)
  uniq (1, 257)     i32  unique sampled token ids (zero-padded, cls first)
"""

import os
import numpy as np

N_CORES = 8
N = 1024
W = 1023          # n - 1
K = 256           # OUTPUT_NUM_TOKENS
K1 = 257          # K + 1 (cls prepended)
NSL = 16          # slices (heads) per b for the sampling prefix
NGS = 8           # gather slices per core
EPS = 1e-6

_CACHE = {}


def _build(stage=4, debug=False):
    import concourse.bacc as bacc
    import concourse.bass as bass
    import concourse.mybir as mybir
    import concourse.tile as tile

    f32 = mybir.dt.float32
    bf16 = mybir.dt.bfloat16
    f16 = mybir.dt.float16
    i32 = mybir.dt.int32
    u32 = mybir.dt.uint32
    Alu = mybir.AluOpType
    Act = mybir.ActivationFunctionType

    nc = bacc.Bacc("TRN2", target_bir_lowering=False, debug=False,
                   num_devices=N_CORES)

    attn_s = nc.dram_tensor("attn_s", [NGS, N, N], f32, kind="ExternalInput")
    value_s = nc.dram_tensor("value_s", [NSL, N, 64], f32, kind="ExternalInput")
    cls_s = nc.dram_tensor("cls_s", [NSL, W], f32, kind="ExternalInput")
    u_s = nc.dram_tensor("u_s", [K, W], f32, kind="ExternalInput")
    out_g = nc.dram_tensor("out", [NGS * K1, N], f32, kind="ExternalOutput")
    uniq_o = nc.dram_tensor("uniq", [1, K1], i32, kind="ExternalOutput")
    if debug:
        dbg_o = nc.dram_tensor("dbg", [128, W], f32, kind="ExternalOutput")

    attn_flat = attn_s.ap().rearrange("s t w -> (s t) w")

    with tile.TileContext(nc) as tc:
        with (
            tc.tile_pool(name="sb", bufs=1) as sb,
            tc.tile_pool(name="sbv", bufs=2) as sbv,
            tc.tile_pool(name="sbu", bufs=2) as sbu,
            tc.tile_pool(name="sbg", bufs=8) as sbg,
            tc.tile_pool(name="ps", bufs=1, space="PSUM") as ps,
        ):
            # ---- input loads first: keep the DMA rings busy from t=0 ----
            # value in 4 flat 1MB chunks (8KB/partition descriptors, full BW).
            # chunk c, partition p, col x: slice = 4c + p//32,
            # token = (p%32)*32 + x//64, d = x%64.
            v_flat = value_s.ap().rearrange("s t d -> (s t d)")
            vchs = []
            for c in range(4):
                vch = sbv.tile([128, 2048], f32, tag="vch", bufs=4)
                eng = nc.sync if c % 2 == 0 else nc.scalar
                eng.dma_start(vch[:], v_flat[c * 262144:(c + 1) * 262144]
                              .rearrange("(p x) -> p x", p=128))
                vchs.append(vch)
            cls_rows = sb.tile([NSL, W], f32)
            nc.scalar.dma_start(cls_rows[:], cls_s.ap())
            u_sbs = []
            for kc in range(2):
                u_sb = sbu.tile([128, W], f32, tag="u")
                eng = nc.sync if kc == 0 else nc.scalar
                eng.dma_start(u_sb[:], u_s.ap()[kc * 128:(kc + 1) * 128])
                u_sbs.append(u_sb)

            # ---- constants ----
            bias9 = sb.tile([128, 1], f32)
            nc.vector.memset(bias9[:], 1e-9)
            bias6 = sb.tile([128, 1], f32)
            nc.vector.memset(bias6[:], EPS)
            ones11_16 = sb.tile([1, 1], f16)
            nc.vector.memset(ones11_16[:], 1.0)
            ones_128_1_16 = sb.tile([128, 1], f16)
            nc.vector.memset(ones_128_1_16[:], 1.0)
            ones_16_128 = sb.tile([16, 128], f16)
            nc.vector.memset(ones_16_128[:], 1.0)
            ones_8_16 = sb.tile([1, 8], f16)
            nc.vector.memset(ones_8_16[:], 1.0)
            iota1024 = sb.tile([128, N], f16)
            nc.gpsimd.iota(iota1024[:], pattern=[[1, N]], base=-1,
                           channel_multiplier=0,
                           allow_small_or_imprecise_dtypes=True)
            iota257 = sb.tile([128, K1], f16)
            nc.gpsimd.iota(iota257[:], pattern=[[1, K1]], base=1,
                           channel_multiplier=0,
                           allow_small_or_imprecise_dtypes=True)
            tvals16 = sb.tile([128, 8], f16)
            nc.gpsimd.iota(tvals16[:], pattern=[[128, 8]], base=0,
                           channel_multiplier=1,
                           allow_small_or_imprecise_dtypes=True)
            rowbase8 = sb.tile([8, 1], f32)
            nc.gpsimd.iota(rowbase8[:], pattern=[[1, 1]], base=0,
                           channel_multiplier=1024,
                           allow_small_or_imprecise_dtypes=True)
            # tok0_mask zeroes each slice's token-0 entropy term: those sit at
            # (p in {0,32,64,96}, col in {0,32,64,96}) of the (128,128) w grid
            tok0_mask = sb.tile([128, NSL * 8], f32)
            nc.vector.memset(tok0_mask[:], 1.0)
            for pp in range(0, 128, 32):
                for cc in range(0, 128, 32):
                    nc.vector.memset(tok0_mask[pp:pp + 1, cc:cc + 1], 0.0)
            # ssel_c[p, s] = 1 iff slice(chunk c, p) == s
            ssels = []
            for c in range(4):
                ssel = sb.tile([128, NSL], f32, name=f"ssel{c}", tag=f"ssel{c}")
                nc.vector.memset(ssel[:], 0.0)
                for h in range(4):
                    nc.vector.memset(
                        ssel[32 * h:32 * h + 32, 4 * c + h:4 * c + h + 1], 1.0)
                ssels.append(ssel)

            # ---- stage 1: value norms -> entropy -> pseudo logits ----
            vn2 = sb.tile([128, NSL * 8], f32)
            for c in range(4):
                vsq = sbv.tile([128, 2048], bf16, tag="vsq")
                if c % 2 == 0:
                    nc.scalar.square(vsq[:], vchs[c][:])
                else:
                    nc.vector.tensor_mul(vsq[:], vchs[c][:], vchs[c][:])
                nc.vector.tensor_reduce(
                    out=vn2[:, c * 32:(c + 1) * 32],
                    in_=vsq[:].rearrange("p (tk d) -> p tk d", d=64),
                    axis=mybir.AxisListType.X, op=Alu.add)
            vn = sb.tile([128, NSL * 8], f32)
            nc.scalar.sqrt(vn[:], vn2[:])
            lnvn = sb.tile([128, NSL * 8], f32)
            nc.scalar.activation(lnvn[:], vn[:], Act.Ln, bias=bias9[:, 0:1])
            w_all = sb.tile([128, NSL * 8], f32)
            nc.vector.tensor_mul(w_all[:], vn[:], lnvn[:])
            wm = sb.tile([128, NSL * 8], f32)
            nc.vector.tensor_mul(wm[:], w_all[:], tok0_mask[:])
            w_red = sb.tile([128, 4], f32)
            nc.vector.tensor_reduce(
                out=w_red[:],
                in_=wm[:].rearrange("p (c j) -> p c j", j=32),
                axis=mybir.AxisListType.X, op=Alu.add)
            # ent_ps = -entropy; the sign cancels in normed = cls/(sum+eps)
            ent_ps = ps.tile([16, 1], f32, tag="A")
            for c in range(4):
                nc.tensor.matmul(out=ent_ps[:], lhsT=ssels[c][:],
                                 rhs=w_red[:, c:c + 1],
                                 start=(c == 0), stop=(c == 3))

            # fp16 weighted: |values| <= ~17k fits fp16 range; the resulting
            # ~2e-4 relative pl error is 12x below the minimum argmax margin,
            # and the fp16 cls matmul is single-pass on PE (f32 is double).
            weighted = sb.tile([NSL, W], f16)
            nc.vector.tensor_scalar(out=weighted[:], in0=cls_rows[:],
                                    scalar1=ent_ps[:], scalar2=None,
                                    op0=Alu.mult)
            # lhsT=ones(16,128) replicates the cls row into all 128 PSUM
            # partitions at no extra PE cost - the later gumbel subtract
            # needs it broadcast anyway.
            cls_ps = ps.tile([128, W], f32, tag="B")
            nc.tensor.matmul(out=cls_ps[:, 0:512], lhsT=ones_16_128[:],
                             rhs=weighted[:, 0:512], start=True, stop=True)
            nc.tensor.matmul(out=cls_ps[:, 512:W], lhsT=ones_16_128[:],
                             rhs=weighted[:, 512:W], start=True, stop=True)
            # S + 1e-6 == S in f32 (|S| ~ 1e8), so the eps add is dropped;
            # normed is fused into the Ln via the per-partition scale operand.
            s_sb = sb.tile([128, 1], f32)
            nc.vector.reduce_sum(out=s_sb[:], in_=cls_ps[:, :],
                                 axis=mybir.AxisListType.X)
            r_sb = sb.tile([128, 1], f32)
            nc.vector.reciprocal(r_sb[:], s_sb[:])
            pl = sb.tile([128, W], f32)
            nc.scalar.activation(pl[:], cls_ps[:, :], Act.Ln,
                                 bias=bias6[:, 0:1], scale=r_sb[:, 0:1])

            if debug and stage == 1:
                nc.sync.dma_start(dbg_o.ap()[0:1, :], pl[0:1, :])

            if stage >= 2:
                # ---- gumbel perturbation + argmax (2 chunks of 128) ----
                ids_f32 = sb.tile([128, 2], f32)
                for kc in range(2):
                    t1 = sbu.tile([128, W], f32, tag="t1")
                    nc.scalar.activation(t1[:], u_sbs[kc][:], Act.Ln,
                                         bias=bias6[:, 0:1])
                    t2 = sbu.tile([128, W], f32, tag="t2")
                    nc.scalar.activation(t2[:], t1[:], Act.Ln,
                                         bias=bias6[:, 0:1], scale=-1.0)
                    pert = sbu.tile([128, W], f32, tag="pert")
                    nc.vector.tensor_sub(pert[:], pl[:], t2[:])
                    mx8 = sbu.tile([128, 8], f32, tag="mx8")
                    nc.vector.max(mx8[:], pert[:])
                    ix8 = sbu.tile([128, 8], u32, tag="ix8")
                    nc.vector.max_index(ix8[:], mx8[:], pert[:])
                    nc.vector.tensor_copy(ids_f32[:, kc:kc + 1], ix8[:, 0:1])

                if debug and stage == 2:
                    nc.sync.dma_start(dbg_o.ap()[:, 0:2], ids_f32[:])

            if stage >= 3:
                # ---- presence counts (onehot matmul, fp16 single-pass) ----
                count_ps = ps.tile([1, N], f32, tag="B")
                for kc in range(2):
                    oh = sbu.tile([128, N], f16, tag="oh")
                    nc.vector.tensor_scalar(out=oh[:], in0=iota1024[:],
                                            scalar1=ids_f32[:, kc:kc + 1],
                                            scalar2=None, op0=Alu.is_equal)
                    for nh in range(2):
                        nc.tensor.matmul(
                            out=count_ps[0:1, nh * 512:(nh + 1) * 512],
                            lhsT=ones_128_1_16[:],
                            rhs=oh[:, nh * 512:(nh + 1) * 512],
                            start=(kc == 0), stop=(kc == 1))

                presence = sb.tile([1, N], f32)
                nc.vector.tensor_scalar(out=presence[:], in0=count_ps[0:1, :],
                                        scalar1=0.0, scalar2=None,
                                        op0=Alu.is_gt)
                pos = sb.tile([1, N], f32)
                nc.vector.tensor_tensor_scan(out=pos[:], data0=presence[:],
                                             data1=presence[:], initial=0.0,
                                             op0=Alu.add, op1=Alu.bypass)
                # q[t] = pos[t]+1 if present else 0; iota257 runs 1..257 so
                # slot j matches value j+1 and 0/absent never matches.
                q16 = sb.tile([1, N], f16)
                nc.vector.scalar_tensor_tensor(out=q16[:], in0=pos[:],
                                               scalar=1.0, in1=presence[:],
                                               op0=Alu.add, op1=Alu.mult)

                # ---- uniq via slot-match matmuls (all fp16, exact) ----
                qt_ps = ps.tile([128, 8], f32, tag="A")
                for tcch in range(8):
                    nc.tensor.matmul(
                        out=qt_ps[:, tcch:tcch + 1],
                        lhsT=q16[0:1, tcch * 128:(tcch + 1) * 128],
                        rhs=ones11_16[:], start=True, stop=True)
                uniq_ps = ps.tile([1, K1], f32, tag="D")
                for tcch in range(8):
                    m_sb = sbu.tile([128, K1], f16, tag="m")
                    nc.vector.tensor_scalar(out=m_sb[:], in0=iota257[:],
                                            scalar1=qt_ps[:, tcch:tcch + 1],
                                            scalar2=None, op0=Alu.is_equal)
                    nc.tensor.matmul(out=uniq_ps[0:1, :],
                                     lhsT=tvals16[:, tcch:tcch + 1],
                                     rhs=m_sb[:],
                                     start=(tcch == 0), stop=(tcch == 7))
                uniq16 = sb.tile([1, K1], f16)
                nc.vector.tensor_copy(uniq16[:], uniq_ps[0:1, :])
                uniq_i32 = sb.tile([1, K1], i32)
                nc.vector.tensor_copy(uniq_i32[:], uniq_ps[0:1, :])
                nc.sync.dma_start(uniq_o.ap(), uniq_i32[:])

            if stage >= 4:
                # ---- gather indices ----
                ujt_ps = ps.tile([128, 2], f32, tag="E")
                for jc in range(2):
                    nc.tensor.matmul(
                        out=ujt_ps[:, jc:jc + 1],
                        lhsT=uniq16[0:1, jc * 128:(jc + 1) * 128],
                        rhs=ones11_16[:], start=True, stop=True)
                idx_sb = sb.tile([128, 2 * NGS], i32)
                for sg in range(NGS):
                    nc.vector.tensor_scalar(out=idx_sb[:, sg * 2:sg * 2 + 2],
                                            in0=ujt_ps[:],
                                            scalar1=float(sg * N),
                                            scalar2=None, op0=Alu.add)
                # broadcast uniq[256] to 8 partitions via rank-1 matmul
                stb_ps = ps.tile([8, 1], f32, tag="A")
                nc.tensor.matmul(out=stb_ps[:], lhsT=ones_8_16[:],
                                 rhs=uniq16[0:1, 256:257],
                                 start=True, stop=True)
                st_idx = sb.tile([8, 1], i32)
                nc.vector.tensor_scalar(out=st_idx[:], in0=stb_ps[:],
                                        scalar1=rowbase8[:], scalar2=None,
                                        op0=Alu.add)

                # ---- the gathers (HW indirect DMA needs (P,1) offsets) ----
                st_tile = sbg.tile([8, N], f32)
                nc.gpsimd.indirect_dma_start(
                    out=st_tile[:],
                    out_offset=None,
                    in_=attn_flat,
                    in_offset=bass.IndirectOffsetOnAxis(ap=st_idx[:], axis=0),
                )
                st_dst = out_g.ap().rearrange("(s j) w -> s j w", j=K1)[:, 256]
                nc.sync.dma_start(st_dst, st_tile[:])
                for sg in range(NGS):
                    for jc in range(2):
                        g_tile = sbg.tile([128, N], f32, tag="g")
                        nc.gpsimd.indirect_dma_start(
                            out=g_tile[:],
                            out_offset=None,
                            in_=attn_flat,
                            in_offset=bass.IndirectOffsetOnAxis(
                                ap=idx_sb[:, sg * 2 + jc:sg * 2 + jc + 1],
                                axis=0),
                        )
                        r0 = sg * K1 + jc * 128
                        eng = nc.sync if (sg * 2 + jc) % 2 == 0 else nc.scalar
                        eng.dma_start(out_g.ap()[r0:r0 + 128], g_tile[:])

    nc.compile()
    return nc


def _get_nc():
    stage = int(os.environ.get("ATS_STAGE", "4"))
    debug = bool(int(os.environ.get("ATS_DEBUG", "0")))
    key = ("nc", stage, debug)
    if key not in _CACHE:
        _CACHE[key] = _build(stage, debug)
    return _CACHE[key]


def kernel(attn, value, u, mask=None):
    from concourse.bass_utils import run_bass_kernel_spmd

    attn = np.ascontiguousarray(attn, dtype=np.float32)
    value = np.ascontiguousarray(value, dtype=np.float32)
    u = np.ascontiguousarray(u, dtype=np.float32)

    nc = _get_nc()
    in_maps = []
    for c in range(N_CORES):
        b, half = c // 2, c % 2
        in_maps.append({
            "attn_s": np.ascontiguousarray(attn[b, half * 8:(half + 1) * 8]),
            "value_s": value[b],
            "cls_s": np.ascontiguousarray(attn[b, :, 0, 1:]),
            "u_s": u[b],
        })
    trace = bool(int(os.environ.get("ATS_TRACE", "0")))
    res = None
    for attempt in range(3):
        try:
            res = run_bass_kernel_spmd(nc, in_maps,
                                       core_ids=list(range(N_CORES)),
                                       trace=trace)
            break
        except Exception:
            # transient NRT/device hiccups (e.g. a prior crashed session
            # left the accelerator recovering) usually clear on retry
            if attempt == 2:
                raise
            import time
            time.sleep(15)
    kernel.last_exec_ns = res.exec_time_ns
    kernel.last_results = res.results

    new_attn = np.empty((4, 16, K1, N), np.float32)
    uniq = np.empty((4, K1), np.int32)
    for c in range(N_CORES):
        b, half = c // 2, c % 2
        new_attn[b, half * 8:(half + 1) * 8] = (
            res.results[c]["out"].reshape(NGS, K1, N))
        if half == 0:
            uniq[b] = res.results[c]["uniq"][0]
    new_mask = uniq != 0
    new_mask[:, 0] = True
    return new_attn, new_mask, uniq


kernel.last_exec_ns = None
kernel.last_results = None
